# revision 1
# baseline (speedup 1.0000x reference)
"""Causal self-attention with RoPE for Trainium2, sharded over 8 NeuronCores.

Sharding (Megatron-style, per the problem's hint):
  8 cores = 4 batches x 2 head-groups (8 of 16 heads each).
  Each core: QKV column-slice projections [1024,512], RoPE, causal attention
  for its 8 heads, and a row-slice output projection producing a partial
  [2048,1024]. Host sums the two partials per batch and adds bo.

Per-core device kernel (Tile framework), all matmuls bf16, fused into a
query-chunk-major loop so projections, attention and the output projection
pipeline through one shared-tag PSUM pool (8 banks exactly):
  per qc: project Q/K/V for 4 t-blocks (lhsT = x^T chunks), RoPE on DVE via
  even/odd strided views, PE-transpose q,k into [c,t] layout, V stored with
  an appended ones column; then scores S^T[j,q] = k^T.T @ q^T (K=64, two
  heads concurrent in row-groups 0/64), exp on ACT (scale folded in),
  causal masks on GPSIMD, AV matmul with M=65 yielding Y^T plus the softmax
  denominator in one accumulation chain, normalization via reciprocal +
  K=1 ones-matmul broadcast; then the output projection for those t-blocks.

No flash-attention running max is needed: scores here are ~N(0, 0.17) and
exp cannot overflow; softmax(x) == softmax(x - max) exactly.
"""
import sys

if "/opt/trn_rl_repo" not in sys.path:
    sys.path.insert(0, "/opt/trn_rl_repo")

from contextlib import ExitStack

import numpy as np
import ml_dtypes

import concourse.bass as bass
import concourse.mybir as mybir
import concourse.tile as tile
from concourse import bacc
from concourse.masks import make_identity

bf16 = ml_dtypes.bfloat16

N_HEAD = 16
ROPE_BASE = 10000.0
B_FULL, T_FULL, C_FULL = 4, 2048, 1024
HD = 64
N_CORES = 8
QCW = 512  # query-chunk width
JBW = 128  # key-block width


def build_core_program(T=T_FULL, HL=8, C=C_FULL, has_bias=False, reps=1,
                       mode="staged", only="abc", tuning=None):
    """Build the per-core Bass program. reps>1 wraps the body in a hardware
    loop (for slope-based timing)."""
    env = {}
    env["T"], env["HL"], env["C"], env["has_bias"] = T, HL, C, has_bias
    env["mode"], env["only"] = mode, only
    env.update(tuning or {})
    env["CL"] = HL * HD
    env["NTB"] = T // 128
    env["NQC"] = T // QCW
    env["NCH"] = env["CL"] // 128
    env["KCH"] = C // 128
    env["NEH"] = C // 512

    f32 = mybir.dt.float32
    b16 = mybir.dt.bfloat16

    nc = bacc.Bacc("TRN2", target_bir_lowering=False, debug=False,
                   enable_asserts=False)

    env["xT"] = nc.dram_tensor("xT", [C, T], b16, kind="ExternalInput").ap()
    env["wq"] = nc.dram_tensor("wq", [C, env["CL"]], b16, kind="ExternalInput").ap()
    env["wk"] = nc.dram_tensor("wk", [C, env["CL"]], b16, kind="ExternalInput").ap()
    env["wv"] = nc.dram_tensor("wv", [C, env["CL"]], b16, kind="ExternalInput").ap()
    env["wo"] = nc.dram_tensor("wo", [env["CL"], C], b16, kind="ExternalInput").ap()
    env["cosd"] = nc.dram_tensor("cosw", [T, 32], f32, kind="ExternalInput").ap()
    env["sind"] = nc.dram_tensor("sinw", [T, 32], f32, kind="ExternalInput").ap()
    env["maskd"] = nc.dram_tensor("masks", [4, JBW, QCW], b16,
                                  kind="ExternalInput").ap()
    env["o"] = nc.dram_tensor("o", [T, C], f32, kind="ExternalOutput").ap()
    names = ["xT", "wq", "wk", "wv", "wo", "cosw", "sinw", "masks"]
    if has_bias:
        env["bqr"] = nc.dram_tensor("bqr", [1, env["CL"]], b16,
                                    kind="ExternalInput").ap()
        env["bkr"] = nc.dram_tensor("bkr", [1, env["CL"]], b16,
                                    kind="ExternalInput").ap()
        env["bvr"] = nc.dram_tensor("bvr", [1, env["CL"]], b16,
                                    kind="ExternalInput").ap()
        names += ["bqr", "bkr", "bvr"]

    with tile.TileContext(nc) as tc:
        with ExitStack() as ctx:
            _body(ctx, tc, env, reps)
    nc.compile()
    return nc, names


def _body(ctx, tc, env, reps):
    nc = tc.nc
    f32 = mybir.dt.float32
    b16 = mybir.dt.bfloat16
    T, HL, C = env["T"], env["HL"], env["C"]
    CL, NTB, NQC, NCH, KCH, NEH = (env["CL"], env["NTB"], env["NQC"],
                                   env["NCH"], env["KCH"], env["NEH"])
    has_bias = env["has_bias"]
    xT, wq, wk, wv, wo = env["xT"], env["wq"], env["wk"], env["wv"], env["wo"]
    cosd, sind, maskd, o = env["cosd"], env["sind"], env["maskd"], env["o"]

    const = ctx.enter_context(tc.tile_pool(name="const", bufs=1))
    persist = ctx.enter_context(tc.tile_pool(name="persist", bufs=1))
    work = ctx.enter_context(tc.tile_pool(name="work", bufs=1))
    pools = {}
    fused = env.get("mode") == "fused"

    def pstile(stage, shape, dt, tag, bufs):
        if fused:
            if tag in ("psqk", "psv", "pst", "bc", "o"):
                tag, bufs = "pj", env.get("pjbufs", 2)
        return pools[stage].tile(shape, dt, tag=tag, bufs=bufs,
                                 name=f"ps_{tag}")

    # ---- constants / weights into SBUF (chunked DMAs -> parallel queues)
    xT_sb = const.tile([128, KCH, T], b16)
    wq_sb = const.tile([128, KCH, CL], b16)
    wk_sb = const.tile([128, KCH, CL], b16)
    wv_sb = const.tile([128, KCH, CL], b16)
    for kc in range(KCH):
        sl = slice(kc * 128, (kc + 1) * 128)
        nc.sync.dma_start(out=xT_sb[:, kc, :], in_=xT[sl, :])
        nc.sync.dma_start(out=wq_sb[:, kc, :], in_=wq[sl, :])
        nc.sync.dma_start(out=wk_sb[:, kc, :], in_=wk[sl, :])
        nc.sync.dma_start(out=wv_sb[:, kc, :], in_=wv[sl, :])
    wo_sb = const.tile([128, NCH, C], b16)
    for cc in range(NCH):
        nc.sync.dma_start(out=wo_sb[:, cc, :],
                          in_=wo[cc * 128:(cc + 1) * 128, :])
    cos_sb = const.tile([128, NTB, 32], f32)
    nc.sync.dma_start(out=cos_sb, in_=cosd.rearrange("(n p) d -> p n d", p=128))
    sin_sb = const.tile([128, NTB, 32], f32)
    nc.sync.dma_start(out=sin_sb, in_=sind.rearrange("(n p) d -> p n d", p=128))
    mask_sb = const.tile([128, 4, QCW], b16)
    nc.sync.dma_start(out=mask_sb, in_=maskd.rearrange("m p q -> p m q"))
    ident = const.tile([128, 128], b16)
    make_identity(nc, ident)
    ones_sb = const.tile([1, 128], b16)
    nc.vector.memset(ones_sb, 1.0)
    if has_bias:
        brows = {}
        for which in ("q", "k", "v"):
            t = const.tile([1, CL], b16, tag=f"b{which}")
            nc.sync.dma_start(out=t, in_=env[f"b{which}r"])
            brows[which] = t

    qT_sb = persist.tile([128, NCH, T], b16)
    kT_sb = persist.tile([128, NCH, T], b16)
    yT_sb = persist.tile([128, NCH, T], b16)
    vaug = persist.tile([128, NTB, HL, 65], b16)
    nc.vector.memset(vaug[:, :, :, 64:65], 1.0)

    def proj(pst, w_sb, tb, which):
        if env.get("abl_noproj"):
            nc.tensor.matmul(pst, xT_sb[:, 0, tb * 128:(tb + 1) * 128],
                             w_sb[:, 0, :], start=True, stop=True)
            return
        for kc in range(KCH):
            nc.tensor.matmul(pst, xT_sb[:, kc, tb * 128:(tb + 1) * 128],
                             w_sb[:, kc, :], start=(kc == 0),
                             stop=(kc == KCH - 1 and not has_bias))
        if has_bias:
            nc.tensor.matmul(pst, ones_sb, brows[which], start=False, stop=True)

    def bchead(t):
        # [128, 32] -> [128, HL, 32] with a step-0 (broadcast) head dim
        return bass.AP(tensor=t.tensor, offset=t.offset,
                       ap=[t.ap[0], [0, HL], t.ap[1]])

    def stage_a(tb):
        cosb = bchead(cos_sb[:, tb, :])
        sinb = bchead(sin_sb[:, tb, :])
        for which, w_sb, dstT in (("q", wq_sb, qT_sb), ("k", wk_sb, kT_sb)):
            psqk = pstile("A", [128, CL], f32, "psqk", env.get("projbufs", 3))
            proj(psqk, w_sb, tb, which)
            x16 = work.tile([128, CL], b16, tag="x16", bufs=3)
            nc.vector.tensor_copy(x16, psqk)
            x4 = x16.rearrange("p (h i two) -> p h i two", two=2, i=32)
            ev, od = x4[:, :, :, 0], x4[:, :, :, 1]
            m1 = work.tile([128, HL, 32], f32, tag="m1", bufs=2)
            m2 = work.tile([128, HL, 32], f32, tag="m2", bufs=2)
            m3 = work.tile([128, HL, 32], f32, tag="m3", bufs=2)
            m4 = work.tile([128, HL, 32], f32, tag="m4", bufs=2)
            rot = work.tile([128, CL], b16, tag="rot", bufs=3)
            if env.get("abl_norope"):
                nc.vector.tensor_copy(rot, x16)
            else:
                nc.vector.tensor_mul(m1, ev, cosb)
                nc.vector.tensor_mul(m2, od, sinb)
                nc.vector.tensor_mul(m3, ev, sinb)
                nc.vector.tensor_mul(m4, od, cosb)
                r4 = rot.rearrange("p (h i two) -> p h i two", two=2, i=32)
                nc.vector.tensor_sub(r4[:, :, :, 0], m1, m2)
                nc.vector.tensor_add(r4[:, :, :, 1], m3, m4)
            if env.get("abl_notrans"):
                nc.vector.tensor_copy(
                    dstT[:, :, tb * 128:(tb + 1) * 128],
                    rot.rearrange("p (cb t) -> p cb t", cb=NCH))
            else:
                pst = pstile("A", [128, CL], b16, "pst", env.get("pstbufs", 2))
                for cb in range(NCH):
                    nc.tensor.transpose(pst[:, cb * 128:(cb + 1) * 128],
                                        rot[:, cb * 128:(cb + 1) * 128], ident)
                nc.vector.tensor_copy(
                    dstT[:, :, tb * 128:(tb + 1) * 128],
                    pst.rearrange("p (cb t) -> p cb t", cb=NCH))
        psv = pstile("A", [128, CL], f32, "psv", env.get("psvbufs", 3))
        proj(psv, wv_sb, tb, "v")
        nc.vector.tensor_copy(vaug[:, tb, :, 0:64],
                              psv.rearrange("p (h d) -> p h d", d=64))

    def stage_b(qc):
        qs = qc * QCW
        njb = (qs + QCW) // JBW
        for g in range(NCH):
            ps_av = [pstile("B", [65, QCW], f32, "av", 3) for _ in range(2)]
            for jb in range(njb):
                ps_s = pstile("B", [128, 2 * QCW], f32, "s", 2)
                if not env.get("abl_noscores"):
                    for hh in range(2):
                        base = hh * 64
                        nc.tensor.matmul(
                            ps_s[:, hh * QCW:(hh + 1) * QCW],
                            kT_sb[base:base + 64, g, jb * JBW:(jb + 1) * JBW],
                            qT_sb[base:base + 64, g, qs:qs + QCW],
                            start=True, stop=True)
                else:
                    nc.vector.memset(ps_s, 0.5)
                e = work.tile([128, 2 * QCW], b16, tag="e", bufs=4)
                if env.get("abl_noexp"):
                    nc.vector.tensor_copy(e, ps_s)
                else:
                    nc.scalar.activation(
                        out=e, in_=ps_s,
                        func=mybir.ActivationFunctionType.Exp,
                        scale=float(1.0 / np.sqrt(HD)))
                if jb >= njb - 4:  # diagonal block: causal mask, both heads
                    m = jb - (njb - 4)
                    mk = mask_sb[:, m, :]
                    mk2 = bass.AP(tensor=mk.tensor, offset=mk.offset,
                                  ap=[mk.ap[0], [0, 2], mk.ap[1]])
                    e2 = e.rearrange("p (two q) -> p two q", two=2)
                    nc.vector.tensor_mul(e2, e2, mk2)
                if not env.get("abl_noav"):
                    for hh in range(2):
                        h = g * 2 + hh
                        nc.tensor.matmul(
                            ps_av[hh], vaug[:, jb, h, :],
                            e[:, hh * QCW:(hh + 1) * QCW],
                            start=(jb == 0), stop=(jb == njb - 1))
            for hh in range(2):
                base = hh * 64
                if env.get("abl_noav"):
                    nc.vector.tensor_copy(yT_sb[base:base + 64, g, qs:qs + QCW],
                                          mask_sb[0:64, 0, :])
                    continue
                rinv = work.tile([1, QCW], f32, tag="rinv", bufs=4)
                nc.vector.reciprocal(rinv, ps_av[hh][64:65, :])
                rb16 = work.tile([1, QCW], b16, tag="rb16", bufs=4)
                nc.vector.tensor_copy(rb16, rinv)
                ps_bc = pstile("B", [64, QCW], f32, "bc", 1)
                nc.tensor.matmul(ps_bc, ones_sb[0:1, 0:64], rb16,
                                 start=True, stop=True)
                rb = work.tile([64, QCW], f32, tag="rb", bufs=4)
                nc.vector.tensor_copy(rb, ps_bc)
                nc.vector.tensor_mul(
                    yT_sb[base:base + 64, g, qs:qs + QCW],
                    ps_av[hh][0:64, :], rb)

    def stage_c(tb):
        for eh in range(NEH):
            ps_o = pstile("C", [128, 512], f32, "o", 4)
            for cc in range(NCH):
                nc.tensor.matmul(ps_o,
                                 yT_sb[:, cc, tb * 128:(tb + 1) * 128],
                                 wo_sb[:, cc, eh * 512:(eh + 1) * 512],
                                 start=(cc == 0), stop=(cc == NCH - 1))
            o_sb = work.tile([128, 512], f32, tag="osb", bufs=3)
            nc.vector.tensor_copy(o_sb, ps_o)
            nc.sync.dma_start(
                out=o[tb * 128:(tb + 1) * 128, eh * 512:(eh + 1) * 512],
                in_=o_sb)

    def body_once():
        if fused:
            with tc.tile_pool(name="psF", bufs=1, space="PSUM") as pF:
                pools["A"] = pools["B"] = pools["C"] = pF
                for qc in range(NQC):
                    for tb in range(qc * 4, qc * 4 + 4):
                        stage_a(tb)
                    stage_b(qc)
                    for tb in range(qc * 4, qc * 4 + 4):
                        stage_c(tb)
            return
        only = env.get("only", "abc")
        if "a" in only:
            with tc.tile_pool(name="psA", bufs=1, space="PSUM") as pA:
                pools["A"] = pA
                for tb in range(NTB):
                    stage_a(tb)
        if env.get("abl_nob"):
            nc.gpsimd.memset(yT_sb, 0.5)
        if "b" in only and not env.get("abl_nob"):
            with tc.tile_pool(name="psB", bufs=1, space="PSUM") as pB:
                pools["B"] = pB
                for qc in range(NQC):
                    stage_b(qc)
        if "c" in only:
            with tc.tile_pool(name="psC", bufs=1, space="PSUM") as pC:
                pools["C"] = pC
                for tb in range(NTB):
                    stage_c(tb)

    if reps == 1:
        body_once()
    else:
        with tc.For_i(0, reps, 1):
            body_once()


def make_host_aux(T=T_FULL):
    """cos/sin caches [T, 32] f32 and causal masks [4, 128, 512] bf16."""
    inv_freq = (1.0 / ROPE_BASE ** (np.arange(0, HD, 2, dtype=np.float32)
                                    / np.float32(HD))).astype(np.float32)
    pos = np.arange(T, dtype=np.float32)
    freqs = np.outer(pos, inv_freq).astype(np.float32)
    cos, sin = np.cos(freqs).astype(np.float32), np.sin(freqs).astype(np.float32)
    jf = np.arange(JBW)[:, None]
    qf = np.arange(QCW)[None, :]
    masks = np.stack([(qf >= m * JBW + jf) for m in range(4)]).astype(bf16)
    return cos, sin, masks


def make_in_maps(x, Wq, bq, Wk, bk, Wv, bv, Wo, T=T_FULL, HL=8):
    """Shard inputs for the 8 cores: core i = (batch i//2, head-group i%2)."""
    CL = HL * HD
    cos, sin, masks = make_host_aux(T)
    B = x.shape[0]
    n_groups = N_CORES // B
    has_bias = bool(np.any(bq) or np.any(bk) or np.any(bv))
    in_maps = []
    for core in range(N_CORES):
        b, g = divmod(core, n_groups)
        cols = slice(g * CL, (g + 1) * CL)
        m = {
            "xT": np.ascontiguousarray(x[b].astype(bf16).T),
            "wq": np.ascontiguousarray(Wq[:, cols].astype(bf16)),
            "wk": np.ascontiguousarray(Wk[:, cols].astype(bf16)),
            "wv": np.ascontiguousarray(Wv[:, cols].astype(bf16)),
            "wo": np.ascontiguousarray(Wo[cols, :].astype(bf16)),
            "cosw": cos, "sinw": sin, "masks": masks,
        }
        if has_bias:
            m["bqr"] = bq[None, cols].astype(bf16)
            m["bkr"] = bk[None, cols].astype(bf16)
            m["bvr"] = bv[None, cols].astype(bf16)
        in_maps.append(m)
    return in_maps, has_bias


_CACHE = {}


def kernel(x, Wq, bq, Wk, bk, Wv, bv, Wo, bo):
    x = np.asarray(x, np.float32)
    B, T, C = x.shape
    assert (B, T, C) == (B_FULL, T_FULL, C_FULL), (B, T, C)
    in_maps, has_bias = make_in_maps(x, Wq, bq, Wk, bk, Wv, bv, Wo)
    key = ("full", has_bias)
    if key not in _CACHE:
        _CACHE[key] = build_core_program(T=T_FULL, HL=8, C=C_FULL,
                                         has_bias=has_bias)
    nc, _names = _CACHE[key]
    from concourse.bass_utils import run_bass_kernel_spmd
    res = run_bass_kernel_spmd(nc, in_maps, core_ids=list(range(N_CORES)),
                               trace=False)
    bo32 = np.asarray(bo, np.float32)
    out = np.empty((B, T, C), np.float32)
    n_groups = N_CORES // B
    for b in range(B):
        acc = res.results[b * n_groups]["o"].astype(np.float32)
        for g in range(1, n_groups):
            acc = acc + res.results[b * n_groups + g]["o"]
        out[b] = acc + bo32[None, :]
    return out



# revision 6
# speedup vs baseline: 1.0946x; 1.0946x over previous
"""Causal self-attention with RoPE for Trainium2, sharded over 8 NeuronCores.

Sharding (Megatron-style): 8 cores = 4 batches x 2 head-groups (8 of 16
heads each). Each core: QKV column-slice projections [1024,512], RoPE,
causal attention for its 8 heads, and a row-slice output projection
producing a partial [2048,1024] (bf16). Host sums the two partials per
batch and adds bo.

Per-core kernel (Tile framework), fused per query-chunk (512 queries):

- Q/K projections run transposed (weight chunks as the stationary
  operand, x^T as the moving operand) so q^T/k^T land directly in
  [c, t] layout -- no PE transposes. Host permutes Wq/Wk columns within
  each head to [evens | odds] (cancels in the q.k dot product), which
  makes RoPE three full-width bf16 DVE ops: m1 = x*cos, u = x*sin_signed,
  rot = m1 + swap32(u), where swap32 (partition p <-> p^32) is done by
  4 small SBUF->SBUF DMAs issued from the idle GPSIMD queue.
- Scores S^T[k,q] = k^T.T @ q^T per head pair in PE row-groups 0/64
  (concurrent in the array). Diagonal blocks compute only the causally
  valid query suffix (N = 512-128m), which also shrinks exp and AV.
- exp on ACT (scale=1/sqrt(hd) folded in); the single lower-triangle
  [128,128] mask is applied only to the partial diagonal sub-block.
- AV: Y^T = V^T @ E for the two heads col-tiled at PSUM partitions 0/64
  (concurrent), accumulated over key blocks; a second all-ones [128,64]
  stationary matmul produces the softmax denominator replicated over all
  64 rows, so normalization is one reciprocal + one multiply covering
  both heads.
- Output projection consumes y^T directly; results DMA out as bf16.

No flash-attention running max is needed: scores are ~N(0, 0.17) and exp
cannot overflow; softmax(x) == softmax(x - max) exactly.
"""
import sys

if "/opt/trn_rl_repo" not in sys.path:
    sys.path.insert(0, "/opt/trn_rl_repo")

from contextlib import ExitStack

import numpy as np
import ml_dtypes

import concourse.bass as bass
import concourse.mybir as mybir
import concourse.tile as tile
from concourse import bacc

bf16 = ml_dtypes.bfloat16

N_HEAD = 16
ROPE_BASE = 10000.0
B_FULL, T_FULL, C_FULL = 4, 2048, 1024
HD = 64
N_CORES = 8
QCW = 512  # query-chunk width
JBW = 128  # key-block width


def build_core_program(T=T_FULL, HL=8, C=C_FULL, has_bias=False, reps=1,
                       tuning=None):
    env = dict(tuning or {})
    env["T"], env["HL"], env["C"], env["has_bias"] = T, HL, C, has_bias
    env["CL"] = HL * HD
    env["NTB"] = T // 128
    env["NQC"] = T // QCW
    env["NCH"] = env["CL"] // 128
    env["KCH"] = C // 128
    env["NEH"] = C // 512

    f32 = mybir.dt.float32
    b16 = mybir.dt.bfloat16

    nc = bacc.Bacc("TRN2", target_bir_lowering=False, debug=False,
                   enable_asserts=False)

    env["xT"] = nc.dram_tensor("xT", [C, T], b16, kind="ExternalInput").ap()
    env["wq"] = nc.dram_tensor("wq", [C, env["CL"]], b16, kind="ExternalInput").ap()
    env["wk"] = nc.dram_tensor("wk", [C, env["CL"]], b16, kind="ExternalInput").ap()
    env["wv"] = nc.dram_tensor("wv", [C, env["CL"]], b16, kind="ExternalInput").ap()
    env["wo"] = nc.dram_tensor("wo", [env["CL"], C], b16, kind="ExternalInput").ap()
    env["cosd"] = nc.dram_tensor("cosct", [128, T], b16, kind="ExternalInput").ap()
    env["sind"] = nc.dram_tensor("sinct", [128, T], b16, kind="ExternalInput").ap()
    env["maskd"] = nc.dram_tensor("maskt", [JBW, JBW], b16,
                                  kind="ExternalInput").ap()
    env["o"] = nc.dram_tensor("o", [T, C], b16, kind="ExternalOutput").ap()
    names = ["xT", "wq", "wk", "wv", "wo", "cosct", "sinct", "maskt"]
    if has_bias:
        env["bqr"] = nc.dram_tensor("bqr", [1, env["CL"]], b16,
                                    kind="ExternalInput").ap()
        env["bkr"] = nc.dram_tensor("bkr", [1, env["CL"]], b16,
                                    kind="ExternalInput").ap()
        env["bvr"] = nc.dram_tensor("bvr", [1, env["CL"]], b16,
                                    kind="ExternalInput").ap()
        names += ["bqr", "bkr", "bvr"]

    with tile.TileContext(nc) as tc:
        with ExitStack() as ctx:
            _body(ctx, tc, env, reps)
    nc.compile()
    return nc, names


def _body(ctx, tc, env, reps):
    nc = tc.nc
    f32 = mybir.dt.float32
    b16 = mybir.dt.bfloat16
    T, HL, C = env["T"], env["HL"], env["C"]
    CL, NTB, NQC, NCH, KCH, NEH = (env["CL"], env["NTB"], env["NQC"],
                                   env["NCH"], env["KCH"], env["NEH"])
    has_bias = env["has_bias"]
    xT, wq, wk, wv, wo = env["xT"], env["wq"], env["wk"], env["wv"], env["wo"]
    cosd, sind, maskd, o = env["cosd"], env["sind"], env["maskd"], env["o"]

    const = ctx.enter_context(tc.tile_pool(name="const", bufs=1))
    persist = ctx.enter_context(tc.tile_pool(name="persist", bufs=1))
    work = ctx.enter_context(tc.tile_pool(name="work", bufs=1))

    # ---- constants / weights into SBUF (chunked DMAs -> parallel queues)
    xT_sb = const.tile([128, KCH, T], b16)
    wq_sb = const.tile([128, KCH, CL], b16)
    wk_sb = const.tile([128, KCH, CL], b16)
    wv_sb = const.tile([128, KCH, CL], b16)
    for kc in range(KCH):
        sl = slice(kc * 128, (kc + 1) * 128)
        nc.sync.dma_start(out=xT_sb[:, kc, :], in_=xT[sl, :])
        nc.sync.dma_start(out=wq_sb[:, kc, :], in_=wq[sl, :])
        nc.sync.dma_start(out=wk_sb[:, kc, :], in_=wk[sl, :])
        nc.sync.dma_start(out=wv_sb[:, kc, :], in_=wv[sl, :])
    wo_sb = const.tile([128, NCH, C], b16)
    for cc in range(NCH):
        nc.sync.dma_start(out=wo_sb[:, cc, :],
                          in_=wo[cc * 128:(cc + 1) * 128, :])
    cos_sb = const.tile([128, T], b16)
    nc.sync.dma_start(out=cos_sb, in_=cosd)
    sin_sb = const.tile([128, T], b16)
    nc.sync.dma_start(out=sin_sb, in_=sind)
    mask_sb = const.tile([128, JBW], b16)
    nc.sync.dma_start(out=mask_sb, in_=maskd)
    ones64 = const.tile([128, 64], b16)
    nc.vector.memset(ones64, 1.0)
    if has_bias:
        onesrow = const.tile([1, QCW], b16)
        nc.vector.memset(onesrow, 1.0)
        brows = {}
        for which in ("q", "k", "v"):
            t = const.tile([1, CL], b16, tag=f"b{which}")
            nc.sync.dma_start(out=t, in_=env[f"b{which}r"])
            brows[which] = t

    kT_sb = persist.tile([128, NCH, T], b16)
    v_sb = persist.tile([128, NTB, HL, 64], b16)

    psum = ctx.enter_context(tc.tile_pool(name="ps", bufs=1, space="PSUM"))

    pjbufs = env.get("pjbufs", 2)
    sbufs = env.get("sbufs", 2)

    def qk_proj_rope(qc, qT_t):
        """Transposed Q/K projection + RoPE for t-chunk qc; writes qT tile
        and kT_sb columns."""
        ts = qc * QCW
        for which, w_sb in (("q", wq_sb), ("k", wk_sb)):
            for cc in range(NCH):
                ps = psum.tile([128, QCW], f32, tag="pj", bufs=pjbufs,
                               name="ps_pj")
                for kc in range(KCH):
                    nc.tensor.matmul(ps, w_sb[:, kc, cc * 128:(cc + 1) * 128],
                                     xT_sb[:, kc, ts:ts + QCW],
                                     start=(kc == 0),
                                     stop=(kc == KCH - 1 and not has_bias))
                if has_bias:
                    nc.tensor.matmul(
                        ps, brows[which][0:1, cc * 128:(cc + 1) * 128],
                        onesrow, start=False, stop=True)
                x16 = work.tile([128, QCW], b16, tag="x16", bufs=3)
                nc.vector.tensor_copy(x16, ps)
                m1 = work.tile([128, QCW], b16, tag="m1", bufs=3)
                nc.vector.tensor_mul(m1, x16, cos_sb[:, ts:ts + QCW])
                us = work.tile([128, QCW], b16, tag="us", bufs=3)
                nc.vector.tensor_mul(us, x16, sin_sb[:, ts:ts + QCW])
                wsw = work.tile([128, QCW], b16, tag="wsw", bufs=3)
                for blk in range(4):
                    sp = blk ^ 1
                    nc.gpsimd.dma_start(
                        out=wsw[blk * 32:(blk + 1) * 32, :],
                        in_=us[sp * 32:(sp + 1) * 32, :])
                dstv = (qT_t[:, cc, :] if which == "q"
                        else kT_sb[:, cc, ts:ts + QCW])
                nc.vector.tensor_add(dstv, m1, wsw)

    def v_proj(tb):
        ps = psum.tile([128, CL], f32, tag="pj", bufs=pjbufs, name="ps_pj")
        for kc in range(KCH):
            nc.tensor.matmul(ps, xT_sb[:, kc, tb * 128:(tb + 1) * 128],
                             wv_sb[:, kc, :], start=(kc == 0),
                             stop=(kc == KCH - 1 and not has_bias))
        if has_bias:
            nc.tensor.matmul(ps, onesrow[:, 0:128], brows["v"],
                             start=False, stop=True)
        nc.vector.tensor_copy(v_sb[:, tb, :, :], ps)

    def attention(qc, qT_t, yT_t):
        qs = qc * QCW
        njb = (qs + QCW) // JBW
        for g in range(NCH):
            ps_av = psum.tile([128, QCW], f32, tag="av", bufs=1, name="ps_av")
            ps_d = psum.tile([128, QCW], f32, tag="d", bufs=1, name="ps_d")
            for jb in range(njb):
                m = jb - (njb - 4)  # >= 0 on diagonal blocks
                off = max(m, 0) * JBW
                ps_s = psum.tile([128, 2 * QCW], f32, tag="s", bufs=sbufs,
                                 name="ps_s")
                for hh in range(2):
                    base = hh * 64
                    nc.tensor.matmul(
                        ps_s[:, hh * QCW + off:(hh + 1) * QCW],
                        kT_sb[base:base + 64, g, jb * JBW:(jb + 1) * JBW],
                        qT_t[base:base + 64, g, off:QCW],
                        start=True, stop=True)
                e = work.tile([128, 2, QCW], b16, tag="e", bufs=3)
                s3 = ps_s.rearrange("p (two q) -> p two q", two=2)
                nc.scalar.activation(
                    out=e[:, :, off:], in_=s3[:, :, off:],
                    func=mybir.ActivationFunctionType.Exp,
                    scale=float(1.0 / np.sqrt(HD)))
                if m >= 0:
                    e2 = e[:, :, off:off + JBW]
                    mk = mask_sb
                    mk2 = bass.AP(tensor=mk.tensor, offset=mk.offset,
                                  ap=[mk.ap[0], [0, 2], mk.ap[1]])
                    nc.vector.tensor_mul(e2, e2, mk2)
                for hh in range(2):
                    h = g * 2 + hh
                    nc.tensor.matmul(
                        ps_av[hh * 64:(hh + 1) * 64, off:],
                        v_sb[:, jb, h, :], e[:, hh, off:],
                        start=(jb == 0), stop=(jb == njb - 1),
                        skip_group_check=(hh == 1))
                for hh in range(2):
                    nc.tensor.matmul(
                        ps_d[hh * 64:(hh + 1) * 64, off:],
                        ones64, e[:, hh, off:],
                        start=(jb == 0), stop=(jb == njb - 1),
                        skip_group_check=(hh == 1))
            rinv = work.tile([128, QCW], f32, tag="rinv", bufs=2)
            nc.vector.reciprocal_approx_fast(out=rinv, in_=ps_d)
            nc.vector.tensor_mul(yT_t[:, g, :], ps_av, rinv)

    def out_proj(qc, yT_t):
        for t4 in range(4):
            tb = qc * 4 + t4
            for eh in range(NEH):
                ps_o = psum.tile([128, 512], f32, tag="pj", bufs=pjbufs,
                                 name="ps_pj")
                for cc in range(NCH):
                    nc.tensor.matmul(ps_o,
                                     yT_t[:, cc, t4 * 128:(t4 + 1) * 128],
                                     wo_sb[:, cc, eh * 512:(eh + 1) * 512],
                                     start=(cc == 0), stop=(cc == NCH - 1))
                osb = work.tile([128, 512], b16, tag="osb", bufs=3)
                nc.vector.tensor_copy(osb, ps_o)
                nc.sync.dma_start(
                    out=o[tb * 128:(tb + 1) * 128, eh * 512:(eh + 1) * 512],
                    in_=osb)

    def body_once():
        for qc in range(NQC):
            qT_t = work.tile([128, NCH, QCW], b16, tag="qT", bufs=2)
            yT_t = work.tile([128, NCH, QCW], b16, tag="yT", bufs=2)
            qk_proj_rope(qc, qT_t)
            for tb in range(qc * 4, qc * 4 + 4):
                v_proj(tb)
            attention(qc, qT_t, yT_t)
            out_proj(qc, yT_t)

    if reps == 1:
        body_once()
    else:
        with tc.For_i(0, reps, 1):
            body_once()


def _qk_perm(HL):
    """Column permutation putting each head's dims in [evens | odds] order."""
    p = []
    for h in range(HL):
        p.extend(h * HD + np.arange(0, HD, 2))
        p.extend(h * HD + np.arange(1, HD, 2))
    return np.asarray(p)


def make_host_aux(T=T_FULL):
    """cos/sin caches [128, T] bf16 (RoPE in [c, t] layout with the ev/od
    split and sign folded into sin) and the [128, 128] lower-triangle mask."""
    inv_freq = (1.0 / ROPE_BASE ** (np.arange(0, HD, 2, dtype=np.float32)
                                    / np.float32(HD))).astype(np.float32)
    pos = np.arange(T, dtype=np.float32)
    p = np.arange(128)
    freqs = np.outer(inv_freq[p % 32], pos)  # [128, T]
    cos = np.cos(freqs).astype(bf16)
    sgn = np.where((p % 64) < 32, 1.0, -1.0).astype(np.float32)
    sin = (np.sin(freqs) * sgn[:, None]).astype(bf16)
    kk = np.arange(JBW)[:, None]
    qq = np.arange(JBW)[None, :]
    mask = (qq >= kk).astype(bf16)
    return cos, sin, mask


def make_in_maps(x, Wq, bq, Wk, bk, Wv, bv, Wo, T=T_FULL, HL=8):
    """Shard inputs for the 8 cores: core i = (batch i//2, head-group i%2)."""
    CL = HL * HD
    cos, sin, mask = make_host_aux(T)
    perm = _qk_perm(HL)
    B = x.shape[0]
    n_groups = N_CORES // B
    has_bias = bool(np.any(bq) or np.any(bk) or np.any(bv))
    in_maps = []
    for core in range(N_CORES):
        b, g = divmod(core, n_groups)
        cols = slice(g * CL, (g + 1) * CL)
        m = {
            "xT": np.ascontiguousarray(x[b].astype(bf16).T),
            "wq": np.ascontiguousarray(Wq[:, cols][:, perm].astype(bf16)),
            "wk": np.ascontiguousarray(Wk[:, cols][:, perm].astype(bf16)),
            "wv": np.ascontiguousarray(Wv[:, cols].astype(bf16)),
            "wo": np.ascontiguousarray(Wo[cols, :].astype(bf16)),
            "cosct": cos, "sinct": sin, "maskt": mask,
        }
        if has_bias:
            m["bqr"] = bq[cols][perm][None, :].astype(bf16)
            m["bkr"] = bk[cols][perm][None, :].astype(bf16)
            m["bvr"] = bv[None, cols].astype(bf16)
        in_maps.append(m)
    return in_maps, has_bias


_CACHE = {}


def kernel(x, Wq, bq, Wk, bk, Wv, bv, Wo, bo):
    x = np.asarray(x, np.float32)
    B, T, C = x.shape
    assert (B, T, C) == (B_FULL, T_FULL, C_FULL), (B, T, C)
    in_maps, has_bias = make_in_maps(x, Wq, bq, Wk, bk, Wv, bv, Wo)
    key = ("full", has_bias)
    if key not in _CACHE:
        _CACHE[key] = build_core_program(T=T_FULL, HL=8, C=C_FULL,
                                         has_bias=has_bias)
    nc, _names = _CACHE[key]
    from concourse.bass_utils import run_bass_kernel_spmd
    res = run_bass_kernel_spmd(nc, in_maps, core_ids=list(range(N_CORES)),
                               trace=False)
    bo32 = np.asarray(bo, np.float32)
    out = np.empty((B, T, C), np.float32)
    n_groups = N_CORES // B
    for b in range(B):
        acc = res.results[b * n_groups]["o"].astype(np.float32)
        for g in range(1, n_groups):
            acc = acc + res.results[b * n_groups + g]["o"].astype(np.float32)
        out[b] = acc + bo32[None, :]
    return out


# revision 8
# speedup vs baseline: 1.5217x; 1.3902x over previous
"""Causal self-attention with RoPE for Trainium2, sharded over 8 NeuronCores.

Sharding (Megatron-style): 8 cores = 4 batches x 2 head-groups (8 of 16
heads each). Each core: QKV column-slice projections [1024,512], RoPE,
causal attention for its 8 heads, and a row-slice output projection
producing a partial [2048,1024] (bf16). Host sums the two partials per
batch and adds bo.

Per-core kernel (Tile framework), fused per query-chunk (512 queries):

- Q/K projections run transposed (weight chunks as the stationary
  operand, x^T as the moving operand) so q^T/k^T land directly in
  [c, t] layout -- no PE transposes. Host permutes Wq/Wk columns within
  each head to [evens | odds] (cancels in the q.k dot product), which
  makes RoPE three full-width bf16 DVE ops: m1 = x*cos, u = x*sin_signed,
  rot = m1 + swap32(u), where swap32 (partition p <-> p^32) is done by
  4 small SBUF->SBUF DMAs issued from the idle GPSIMD queue.
- Scores S^T[k,q] = k^T.T @ q^T per head pair in PE row-groups 0/64
  (concurrent in the array). Diagonal blocks compute only the causally
  valid query suffix (N = 512-128m), which also shrinks exp and AV.
- exp on ACT (scale=1/sqrt(hd) folded in); the single lower-triangle
  [128,128] mask is applied only to the partial diagonal sub-block.
- AV: Y^T = V^T @ E for the two heads col-tiled at PSUM partitions 0/64
  (concurrent), accumulated over key blocks; a second all-ones [128,64]
  stationary matmul produces the softmax denominator replicated over all
  64 rows, so normalization is one reciprocal + one multiply covering
  both heads.
- Output projection consumes y^T directly; results DMA out as bf16.

No flash-attention running max is needed: scores are ~N(0, 0.17) and exp
cannot overflow; softmax(x) == softmax(x - max) exactly.
"""
import sys

if "/opt/trn_rl_repo" not in sys.path:
    sys.path.insert(0, "/opt/trn_rl_repo")

from contextlib import ExitStack

import numpy as np
import ml_dtypes

import concourse.bass as bass
import concourse.mybir as mybir
import concourse.tile as tile
from concourse import bacc

bf16 = ml_dtypes.bfloat16

N_HEAD = 16
ROPE_BASE = 10000.0
B_FULL, T_FULL, C_FULL = 4, 2048, 1024
HD = 64
N_CORES = 8
QCW = 512  # query-chunk width
JBW = 128  # key-block width


def build_core_program(T=T_FULL, HL=8, C=C_FULL, has_bias=False, reps=1,
                       tuning=None):
    env = dict(tuning or {})
    env["T"], env["HL"], env["C"], env["has_bias"] = T, HL, C, has_bias
    env["CL"] = HL * HD
    env["NTB"] = T // 128
    env["NQC"] = T // QCW
    env["NCH"] = env["CL"] // 128
    env["KCH"] = C // 128
    env["NEH"] = C // 512

    f32 = mybir.dt.float32
    b16 = mybir.dt.bfloat16

    nc = bacc.Bacc("TRN2", target_bir_lowering=False, debug=False,
                   enable_asserts=False)

    env["xT"] = nc.dram_tensor("xT", [C, T], b16, kind="ExternalInput").ap()
    env["wq"] = nc.dram_tensor("wq", [C, env["CL"]], b16, kind="ExternalInput").ap()
    env["wk"] = nc.dram_tensor("wk", [C, env["CL"]], b16, kind="ExternalInput").ap()
    env["wv"] = nc.dram_tensor("wv", [C, env["CL"]], b16, kind="ExternalInput").ap()
    env["wo"] = nc.dram_tensor("wo", [env["CL"], C], b16, kind="ExternalInput").ap()
    env["cosd"] = nc.dram_tensor("cosct", [128, T], b16, kind="ExternalInput").ap()
    env["sind"] = nc.dram_tensor("sinct", [128, T], b16, kind="ExternalInput").ap()
    env["maskd"] = nc.dram_tensor("maskt", [JBW, JBW], b16,
                                  kind="ExternalInput").ap()
    env["o"] = nc.dram_tensor("o", [T, C], b16, kind="ExternalOutput").ap()
    names = ["xT", "wq", "wk", "wv", "wo", "cosct", "sinct", "maskt"]
    if has_bias:
        env["bqr"] = nc.dram_tensor("bqr", [1, env["CL"]], b16,
                                    kind="ExternalInput").ap()
        env["bkr"] = nc.dram_tensor("bkr", [1, env["CL"]], b16,
                                    kind="ExternalInput").ap()
        env["bvr"] = nc.dram_tensor("bvr", [1, env["CL"]], b16,
                                    kind="ExternalInput").ap()
        names += ["bqr", "bkr", "bvr"]

    with tile.TileContext(nc) as tc:
        with ExitStack() as ctx:
            _body(ctx, tc, env, reps)
    nc.compile()
    return nc, names


def _body(ctx, tc, env, reps):
    nc = tc.nc
    f32 = mybir.dt.float32
    b16 = mybir.dt.bfloat16
    T, HL, C = env["T"], env["HL"], env["C"]
    CL, NTB, NQC, NCH, KCH, NEH = (env["CL"], env["NTB"], env["NQC"],
                                   env["NCH"], env["KCH"], env["NEH"])
    has_bias = env["has_bias"]
    xT, wq, wk, wv, wo = env["xT"], env["wq"], env["wk"], env["wv"], env["wo"]
    cosd, sind, maskd, o = env["cosd"], env["sind"], env["maskd"], env["o"]

    const = ctx.enter_context(tc.tile_pool(name="const", bufs=1))
    persist = ctx.enter_context(tc.tile_pool(name="persist", bufs=1))
    work = ctx.enter_context(tc.tile_pool(name="work", bufs=1))

    # ---- constants / weights into SBUF (chunked DMAs -> parallel queues)
    xT_sb = const.tile([128, KCH, T], b16)
    wq_sb = const.tile([128, KCH, CL], b16)
    wk_sb = const.tile([128, KCH, CL], b16)
    wv_sb = const.tile([128, KCH, CL], b16)
    for kc in range(KCH):
        sl = slice(kc * 128, (kc + 1) * 128)
        nc.sync.dma_start(out=xT_sb[:, kc, :], in_=xT[sl, :])
        nc.sync.dma_start(out=wq_sb[:, kc, :], in_=wq[sl, :])
        nc.sync.dma_start(out=wk_sb[:, kc, :], in_=wk[sl, :])
        nc.sync.dma_start(out=wv_sb[:, kc, :], in_=wv[sl, :])
    wo_sb = const.tile([128, NCH, C], b16)
    for cc in range(NCH):
        nc.sync.dma_start(out=wo_sb[:, cc, :],
                          in_=wo[cc * 128:(cc + 1) * 128, :])
    cos_sb = const.tile([128, T], b16)
    nc.sync.dma_start(out=cos_sb, in_=cosd)
    sin_sb = const.tile([128, T], b16)
    nc.sync.dma_start(out=sin_sb, in_=sind)
    mask_sb = const.tile([128, JBW], b16)
    nc.sync.dma_start(out=mask_sb, in_=maskd)
    ones64 = const.tile([128, 64], b16)
    nc.vector.memset(ones64, 1.0)
    if has_bias:
        onesrow = const.tile([1, QCW], b16)
        nc.vector.memset(onesrow, 1.0)
        brows = {}
        for which in ("q", "k", "v"):
            t = const.tile([1, CL], b16, tag=f"b{which}")
            nc.sync.dma_start(out=t, in_=env[f"b{which}r"])
            brows[which] = t

    kT_sb = persist.tile([128, NCH, T], b16)
    v_sb = persist.tile([128, NTB, HL, 64], b16)

    psum = ctx.enter_context(tc.tile_pool(name="ps", bufs=1, space="PSUM"))

    pjbufs = env.get("pjbufs", 2)
    sbufs = env.get("sbufs", 2)

    def qk_proj_rope(qc, qT_t):
        """Transposed Q/K projection + RoPE for t-chunk qc; writes qT tile
        and kT_sb columns."""
        ts = qc * QCW
        for which, w_sb in (("q", wq_sb), ("k", wk_sb)):
            for cc in range(NCH):
                ps = psum.tile([128, QCW], f32, tag="pj", bufs=pjbufs,
                               name="ps_pj")
                for kc in range(KCH):
                    nc.tensor.matmul(ps, w_sb[:, kc, cc * 128:(cc + 1) * 128],
                                     xT_sb[:, kc, ts:ts + QCW],
                                     start=(kc == 0),
                                     stop=(kc == KCH - 1 and not has_bias))
                if has_bias:
                    nc.tensor.matmul(
                        ps, brows[which][0:1, cc * 128:(cc + 1) * 128],
                        onesrow, start=False, stop=True)
                x16 = work.tile([128, QCW], b16, tag="x16", bufs=4)
                nc.vector.tensor_copy(x16, ps)
                m1 = work.tile([128, QCW], b16, tag="m1", bufs=4)
                nc.vector.tensor_mul(m1, x16, cos_sb[:, ts:ts + QCW])
                us = work.tile([128, QCW], b16, tag="us", bufs=4)
                nc.vector.tensor_mul(us, x16, sin_sb[:, ts:ts + QCW])
                wsw = work.tile([128, QCW], b16, tag="wsw", bufs=4)
                for blk in range(4):
                    sp = blk ^ 1
                    nc.gpsimd.dma_start(
                        out=wsw[blk * 32:(blk + 1) * 32, :],
                        in_=us[sp * 32:(sp + 1) * 32, :])
                dstv = (qT_t[:, cc, :] if which == "q"
                        else kT_sb[:, cc, ts:ts + QCW])
                nc.vector.tensor_add(dstv, m1, wsw)

    def v_proj(tb):
        ps = psum.tile([128, CL], f32, tag="pj", bufs=pjbufs, name="ps_pj")
        for kc in range(KCH):
            nc.tensor.matmul(ps, xT_sb[:, kc, tb * 128:(tb + 1) * 128],
                             wv_sb[:, kc, :], start=(kc == 0),
                             stop=(kc == KCH - 1 and not has_bias))
        if has_bias:
            nc.tensor.matmul(ps, onesrow[:, 0:128], brows["v"],
                             start=False, stop=True)
        nc.vector.tensor_copy(v_sb[:, tb, :, :], ps)

    def attention(qc, qT_t, yT_t):
        qs = qc * QCW
        njb = (qs + QCW) // JBW
        for g in range(NCH):
            ps_av = psum.tile([128, QCW], f32, tag="av", bufs=1, name="ps_av")
            ps_d = psum.tile([128, QCW], f32, tag="d", bufs=1, name="ps_d")
            for jb in range(njb):
                m = jb - (njb - 4)  # >= 0 on diagonal blocks
                off = max(m, 0) * JBW
                ps_s = psum.tile([128, 2 * QCW], f32, tag="s", bufs=sbufs,
                                 name="ps_s")
                for hh in range(2):
                    base = hh * 64
                    nc.tensor.matmul(
                        ps_s[:, hh * QCW + off:(hh + 1) * QCW],
                        kT_sb[base:base + 64, g, jb * JBW:(jb + 1) * JBW],
                        qT_t[base:base + 64, g, off:QCW],
                        start=True, stop=True)
                e = work.tile([128, 2, QCW], b16, tag="e", bufs=4)
                s3 = ps_s.rearrange("p (two q) -> p two q", two=2)
                nc.scalar.activation(
                    out=e[:, :, off:], in_=s3[:, :, off:],
                    func=mybir.ActivationFunctionType.Exp,
                    scale=float(1.0 / np.sqrt(HD)))
                if m >= 0:
                    e2 = e[:, :, off:off + JBW]
                    mk = mask_sb
                    mk2 = bass.AP(tensor=mk.tensor, offset=mk.offset,
                                  ap=[mk.ap[0], [0, 2], mk.ap[1]])
                    nc.vector.tensor_mul(e2, e2, mk2)
                for hh in range(2):
                    h = g * 2 + hh
                    nc.tensor.matmul(
                        ps_av[hh * 64:(hh + 1) * 64, off:],
                        v_sb[:, jb, h, :], e[:, hh, off:],
                        start=(jb == 0), stop=(jb == njb - 1),
                        skip_group_check=(hh == 1))
                for hh in range(2):
                    nc.tensor.matmul(
                        ps_d[hh * 64:(hh + 1) * 64, off:],
                        ones64, e[:, hh, off:],
                        start=(jb == 0), stop=(jb == njb - 1),
                        skip_group_check=(hh == 1))
            rinv = work.tile([128, QCW], f32, tag="rinv", bufs=2)
            nc.vector.reciprocal_approx_fast(out=rinv, in_=ps_d)
            nc.vector.tensor_mul(yT_t[:, g, :], ps_av, rinv)

    def out_proj(qc, yT_t):
        for t4 in range(4):
            tb = qc * 4 + t4
            for eh in range(NEH):
                ps_o = psum.tile([128, 512], f32, tag="pj", bufs=pjbufs,
                                 name="ps_pj")
                for cc in range(NCH):
                    nc.tensor.matmul(ps_o,
                                     yT_t[:, cc, t4 * 128:(t4 + 1) * 128],
                                     wo_sb[:, cc, eh * 512:(eh + 1) * 512],
                                     start=(cc == 0), stop=(cc == NCH - 1))
                osb = work.tile([128, 512], b16, tag="osb", bufs=3)
                nc.vector.tensor_copy(osb, ps_o)
                nc.sync.dma_start(
                    out=o[tb * 128:(tb + 1) * 128, eh * 512:(eh + 1) * 512],
                    in_=osb)

    def body_once():
        qT_t = work.tile([128, NCH, QCW], b16, tag="qT", bufs=2)
        qk_proj_rope(0, qT_t)
        for tb in range(4):
            v_proj(tb)
        for qc in range(NQC):
            yT_t = work.tile([128, NCH, QCW], b16, tag="yT", bufs=2)
            attention(qc, qT_t, yT_t)
            if qc + 1 < NQC:
                qT_t = work.tile([128, NCH, QCW], b16, tag="qT", bufs=2)
                qk_proj_rope(qc + 1, qT_t)
                for tb in range((qc + 1) * 4, (qc + 2) * 4):
                    v_proj(tb)
            out_proj(qc, yT_t)

    if reps == 1:
        body_once()
    else:
        with tc.For_i(0, reps, 1):
            body_once()


def _qk_perm(HL):
    """Column permutation putting each head's dims in [evens | odds] order."""
    p = []
    for h in range(HL):
        p.extend(h * HD + np.arange(0, HD, 2))
        p.extend(h * HD + np.arange(1, HD, 2))
    return np.asarray(p)


def make_host_aux(T=T_FULL):
    """cos/sin caches [128, T] bf16 (RoPE in [c, t] layout with the ev/od
    split and sign folded into sin) and the [128, 128] lower-triangle mask."""
    inv_freq = (1.0 / ROPE_BASE ** (np.arange(0, HD, 2, dtype=np.float32)
                                    / np.float32(HD))).astype(np.float32)
    pos = np.arange(T, dtype=np.float32)
    p = np.arange(128)
    freqs = np.outer(inv_freq[p % 32], pos)  # [128, T]
    cos = np.cos(freqs).astype(bf16)
    sgn = np.where((p % 64) < 32, 1.0, -1.0).astype(np.float32)
    sin = (np.sin(freqs) * sgn[:, None]).astype(bf16)
    kk = np.arange(JBW)[:, None]
    qq = np.arange(JBW)[None, :]
    mask = (qq >= kk).astype(bf16)
    return cos, sin, mask


def make_in_maps(x, Wq, bq, Wk, bk, Wv, bv, Wo, T=T_FULL, HL=8):
    """Shard inputs for the 8 cores: core i = (batch i//2, head-group i%2)."""
    CL = HL * HD
    cos, sin, mask = make_host_aux(T)
    perm = _qk_perm(HL)
    B = x.shape[0]
    n_groups = N_CORES // B
    has_bias = bool(np.any(bq) or np.any(bk) or np.any(bv))
    in_maps = []
    for core in range(N_CORES):
        b, g = divmod(core, n_groups)
        cols = slice(g * CL, (g + 1) * CL)
        m = {
            "xT": np.ascontiguousarray(x[b].astype(bf16).T),
            "wq": np.ascontiguousarray(Wq[:, cols][:, perm].astype(bf16)),
            "wk": np.ascontiguousarray(Wk[:, cols][:, perm].astype(bf16)),
            "wv": np.ascontiguousarray(Wv[:, cols].astype(bf16)),
            "wo": np.ascontiguousarray(Wo[cols, :].astype(bf16)),
            "cosct": cos, "sinct": sin, "maskt": mask,
        }
        if has_bias:
            m["bqr"] = bq[cols][perm][None, :].astype(bf16)
            m["bkr"] = bk[cols][perm][None, :].astype(bf16)
            m["bvr"] = bv[None, cols].astype(bf16)
        in_maps.append(m)
    return in_maps, has_bias


_CACHE = {}


def kernel(x, Wq, bq, Wk, bk, Wv, bv, Wo, bo):
    x = np.asarray(x, np.float32)
    B, T, C = x.shape
    assert (B, T, C) == (B_FULL, T_FULL, C_FULL), (B, T, C)
    in_maps, has_bias = make_in_maps(x, Wq, bq, Wk, bk, Wv, bv, Wo)
    key = ("full", has_bias)
    if key not in _CACHE:
        _CACHE[key] = build_core_program(T=T_FULL, HL=8, C=C_FULL,
                                         has_bias=has_bias)
    nc, _names = _CACHE[key]
    from concourse.bass_utils import run_bass_kernel_spmd
    res = run_bass_kernel_spmd(nc, in_maps, core_ids=list(range(N_CORES)),
                               trace=False)
    bo32 = np.asarray(bo, np.float32)
    out = np.empty((B, T, C), np.float32)
    n_groups = N_CORES // B
    for b in range(B):
        acc = res.results[b * n_groups]["o"].astype(np.float32)
        for g in range(1, n_groups):
            acc = acc + res.results[b * n_groups + g]["o"].astype(np.float32)
        out[b] = acc + bo32[None, :]
    return out
